# revision 1
# baseline (speedup 1.0000x reference)
"""Trainium2 Bass kernel: 4096x4096 fp32 image, 16x16 valid cross-correlation + bias.

Strategy: shard output rows across 8 NeuronCores (spatial parallel, halo rows
overlapped host-side). Per core, the conv is computed as banded matmuls on
TensorE: for each kernel column b, a [128 x 113] banded stationary matrix
(band = weight[:, b]) is multiplied by a 128-row strip of X whose free-dim AP
is shifted by b; the 16 matmuls accumulate in PSUM. The band gives each output
row its 16 kernel-row taps; the free-dim shift gives the kernel-column taps.

Matmul dtype is float32r (fp32 storage, ~13-bit-mantissa multiplies, fp32
accumulate): 4x faster than native fp32 matmul, ~1e-4 relative error.
"""
import os
import numpy as np

import concourse.mybir as mybir
import concourse.tile as tile
from concourse import bacc
from concourse.bass_utils import run_bass_kernel_spmd

H = 4096
W = 4096
KH = 16
KW = 16
OH = H - KH + 1  # 4081
OW = W - KW + 1  # 4081
NCORES = 8

RPC = 512  # output rows per core (8*512 = 4096 >= 4081; tail trimmed)
M_TILE = 113  # output rows per matmul pass (128 - KH + 1)
N_TILE = 512  # output cols per matmul (fp32 PSUM bank limit)
STRIPS = [0, 113, 226, 339, 452]  # strip start rows within a core's block
IN_ROWS = STRIPS[-1] + 128  # 580 input rows needed per core

_DT_NAME = os.environ.get("CONV_MM_DTYPE", "float32r")

_build_cache = {}


def _build(dt_name):
    repeat = int(os.environ.get("CONV_REPEAT", "1"))
    loop = int(os.environ.get("CONV_LOOP", "1"))  # hw For_i loop (bench only)
    psum_bufs = int(os.environ.get("CONV_PSUM_BUFS", "8"))
    strip_bufs = int(os.environ.get("CONV_STRIP_BUFS", "3"))
    chunked = int(os.environ.get("CONV_CHUNK_DMA", "1"))
    key = (dt_name, repeat, loop, psum_bufs, strip_bufs, chunked)
    if key in _build_cache:
        return _build_cache[key]
    DT = getattr(mybir.dt, dt_name)
    nc = bacc.Bacc()
    x_d = nc.dram_tensor("Xs", [IN_ROWS, W], DT, kind="ExternalInput")
    wb_d = nc.dram_tensor("wband", [128, KW, M_TILE], DT, kind="ExternalInput")
    bias_d = nc.dram_tensor("biasb", [128, 1], mybir.dt.float32, kind="ExternalInput")
    out_d = nc.dram_tensor("out", [RPC, OW], mybir.dt.float32, kind="ExternalOutput")

    with tile.TileContext(nc) as tc:
        with (
            tc.tile_pool(name="const", bufs=1) as cpool,
            tc.tile_pool(name="strip", bufs=strip_bufs) as spool,
            tc.tile_pool(name="obuf", bufs=3) as opool,
            tc.tile_pool(name="psum", bufs=psum_bufs, space="PSUM") as ppool,
        ):
            wb = cpool.tile([128, KW, M_TILE], DT)
            if chunked:
                # load the b=0 slice first so the first matmul isn't gated on
                # the whole banded-weight tensor; bulk goes on the SWDGE queue
                # so it doesn't queue ahead of the first strip chunk
                nc.sync.dma_start(wb[:, 0:1, :], wb_d[:, 0:1, :])
                nc.gpsimd.dma_start(wb[:, 1:, :], wb_d[:, 1:, :])
                bias_sb = cpool.tile([128, 1], mybir.dt.float32)
                nc.gpsimd.dma_start(bias_sb[:], bias_d[:])
            else:
                nc.sync.dma_start(wb[:], wb_d[:])
                bias_sb = cpool.tile([128, 1], mybir.dt.float32)
                nc.sync.dma_start(bias_sb[:], bias_d[:])

            # fp32r matmuls need an even PSUM free size: tile the columns in
            # uniform N_TILE-wide matmuls; the ragged last tile is computed at
            # n0 = OW - N_TILE and only its non-overlapping tail is stored.
            n0_list = [(n0, 0) for n0 in range(0, OW - N_TILE + 1, N_TILE)]
            covered = (OW // N_TILE) * N_TILE
            if covered < OW:
                n0_list.append((OW - N_TILE, covered - (OW - N_TILE)))

            # strip DMA split points: chunk j ends where N-tile j's reads end,
            # so tile j's matmuls only wait on chunks <= j (PE starts after
            # ~260KB instead of the full 2MB strip)
            chunk_bounds = [0]
            for j in range(len(n0_list)):
                chunk_bounds.append(min((j + 1) * N_TILE + KW - 1, W))

            def body():
                first = True
                for m0 in [s for _ in range(repeat) for s in STRIPS]:
                    mt = min(M_TILE, RPC - m0)
                    strip = spool.tile([128, W], DT, tag="strip")
                    if chunked and first:
                        # chunk only the first strip: its DMA gates the very
                        # first matmuls; later strips prefetch under compute
                        for lo, hi in zip(chunk_bounds, chunk_bounds[1:]):
                            if hi > lo:
                                nc.sync.dma_start(
                                    strip[:, lo:hi], x_d[m0 : m0 + 128, lo:hi]
                                )
                    else:
                        nc.sync.dma_start(strip[:], x_d[m0 : m0 + 128, :])
                    first = False
                    for n0, skip in n0_list:
                        ps = ppool.tile([M_TILE, N_TILE], mybir.dt.float32, tag="ps")
                        for b in range(KW):
                            nc.tensor.matmul(
                                ps[:mt, :N_TILE],
                                wb[:, b, :mt],
                                strip[:, n0 + b : n0 + b + N_TILE],
                                start=(b == 0),
                                stop=(b == KW - 1),
                            )
                        nt = N_TILE - skip
                        ot = opool.tile([M_TILE, N_TILE], mybir.dt.float32, tag="ot")
                        nc.vector.tensor_scalar_add(
                            ot[:mt, :nt], ps[:mt, skip:N_TILE], bias_sb[:mt]
                        )
                        nc.sync.dma_start(
                            out_d[m0 : m0 + mt, n0 + skip : n0 + N_TILE], ot[:mt, :nt]
                        )

            if loop > 1:
                with tc.For_i(0, loop, 1):
                    body()
            else:
                body()
    nc.finalize()
    _build_cache[key] = nc
    return nc


def _run(X, weight, bias, dt_name, trace=False):
    nc = _build(dt_name)
    np_dt = mybir.dt.np(getattr(mybir.dt, dt_name))

    pad_rows = NCORES * RPC + (IN_ROWS - RPC)  # 4164
    Xpad = np.zeros((pad_rows, W), dtype=np_dt)
    Xpad[:H] = X.astype(np_dt)

    # wband[k, b, m] = weight[k - m, b] for 0 <= k-m < KH else 0
    wband = np.zeros((128, KW, M_TILE), dtype=np_dt)
    wc = weight.astype(np_dt)
    for m in range(M_TILE):
        wband[m : m + KH, :, m] = wc
    biasb = np.full((128, 1), np.float32(bias[0]), dtype=np.float32)

    in_maps = [
        {
            "Xs": np.ascontiguousarray(Xpad[c * RPC : c * RPC + IN_ROWS]),
            "wband": wband,
            "biasb": biasb,
        }
        for c in range(NCORES)
    ]
    res = run_bass_kernel_spmd(
        nc, in_maps, core_ids=list(range(NCORES)), trace=trace
    )
    out = np.concatenate([res.results[c]["out"] for c in range(NCORES)], axis=0)
    return out[:OH], res


def kernel(X, weight, bias):
    X = np.asarray(X, dtype=np.float32)
    weight = np.asarray(weight, dtype=np.float32)
    bias = np.asarray(bias, dtype=np.float32)
    out, _ = _run(X, weight, bias, _DT_NAME, trace=False)
    return out

